# revision 55
# baseline (speedup 1.0000x reference)
"""CrossWinAttention Trainium2 kernel.

Data-parallel over the 128 (batch, window) pairs: 16 per NeuronCore x 8 cores.
Each core runs LN + QKV projection + 4-head attention + view-mean + output
projection + skip-add for its pairs.

Key layout/engine choices (v2):
  - x loaded bf16, packed q/k/v in one DMA; token chunk mapping t = 3p + c so
    DRAM runs stay >= 512B.
  - token-major LN (bn_stats/bn_aggr on DVE, rsqrt bit-trick + Newton), xhat
    in bf16 via DVE 4x tensor_scalar.
  - PE transposes with a bf16 identity (1 cyc/col) into bf16 PSUM tiles;
    PSUM->SBUF copies ride the DVE 2x path.
  - q/k projections feature-major (moving xhatT bf16, ap=384); v projection
    token-major (moving wv bf16).
  - scoresT[k,q] per head, fp32r, 2-head groups double-buffered; exp on ACT
    (the only exp-capable engine) straight out of PSUM, bf16 out.
  - AV restructured q-major: out[q, 33] per (qchunk, head) with att as the
    stationary and vp||ones as the moving operand; column 32 accumulates the
    softmax denominator Z for free.  36 matmuls x 33 cols replaces the old
    4608+4608 column AV+denominator scheme.
  - normalization av*(1/Z) with per-partition scalars (q-major makes Z a
    per-partition value), view-mean as a matmul against a 0/1 S-matrix
    (absorbs the token permutation), output projection on [HD,64].
Biases: LN gammas and the attention scale fold into the weights on the host;
1/6 view-mean folds into wp; k bias adds per-partition; v/out biases fold into
skip.  The q-bias softmax cross term is emitted only when nonzero.
"""

import numpy as np
import ml_dtypes
from contextlib import ExitStack

import concourse.bass as bass
import concourse.tile as tile
from concourse import bacc, mybir
from concourse.bass_utils import run_bass_kernel_spmd

# Problem dims (hardcoded per contest rules).
B, N, X, Y, W1, W2, D = 2, 6, 8, 8, 8, 8, 128
H, DH = 4, 32
HD = H * DH
L = X * Y                  # 64 windows
Q = N * W1 * W2            # 384 tokens per window
BL = B * L                 # 128 (b,l) pairs
NCORES = 8
PER_CORE = BL // NCORES    # 16
NW = W1 * W2               # 64
NC = Q // 128              # 3 token chunks
EPS = 1e-5
F32 = mybir.dt.float32
BF16 = mybir.dt.bfloat16
FR = mybir.dt.float32r
I32 = mybir.dt.int32

_COMPILED = {}
NEWTON_ITERS = 1


def _emit(nc, with_bk, with_cross):
    x_d = nc.dram_tensor("X", [PER_CORE, 3, Q, D], BF16, kind="ExternalInput").ap()
    skip_d = nc.dram_tensor("skipT", [PER_CORE, D, NW], F32, kind="ExternalInput").ap()
    wq_d = nc.dram_tensor("wq", [D, HD], BF16, kind="ExternalInput").ap()
    wk_d = nc.dram_tensor("wk", [D, HD], BF16, kind="ExternalInput").ap()
    wv_d = nc.dram_tensor("wv", [D, HD], BF16, kind="ExternalInput").ap()
    wp_d = nc.dram_tensor("wp", [HD, D], FR, kind="ExternalInput").ap()
    s_d = nc.dram_tensor("S", [128, NC, NW], BF16, kind="ExternalInput").ap()
    id_d = nc.dram_tensor("ident", [128, 128], BF16, kind="ExternalInput").ap()
    bk_d = nc.dram_tensor("bk", [HD, 1], F32, kind="ExternalInput").ap()
    u_d = nc.dram_tensor("U", [D, H], BF16, kind="ExternalInput").ap()
    g_d = nc.dram_tensor("gam", [H, 1], F32, kind="ExternalInput").ap()
    out_d = nc.dram_tensor("out", [PER_CORE, D, NW], F32, kind="ExternalOutput").ap()

    Exp = mybir.ActivationFunctionType.Exp
    Alu = mybir.AluOpType

    with tile.TileContext(nc) as tc, ExitStack() as ctx:
        const = ctx.enter_context(tc.tile_pool(name="const", bufs=1))
        sb = ctx.enter_context(tc.tile_pool(name="sb", bufs=4))
        attp = ctx.enter_context(tc.tile_pool(name="attp", bufs=18))
        # PSUM: pa(2) + sc(4) + av(2) = 8 banks exactly
        pa = ctx.enter_context(tc.tile_pool(name="pa", bufs=3, space="PSUM"))
        scp = ctx.enter_context(tc.tile_pool(name="scp", bufs=2, space="PSUM"))
        avp = ctx.enter_context(tc.tile_pool(name="avp", bufs=1, space="PSUM"))

        def cload(name, ap_, shape, dt_):
            t = const.tile(shape, dt_, tag=name, name=name)
            nc.sync.dma_start(t[:], ap_[:])
            return t

        def load_pair(bl):
            x = sb.tile([128, 3, NC, D], BF16, tag="x", name=f"x{bl}", bufs=6)
            nc.sync.dma_start(x[:], x_d[bl].rearrange("t (p c) d -> p t c d", p=128))
            skip = sb.tile([D, NW], F32, tag="skip", name=f"skip{bl}", bufs=6)
            nc.sync.dma_start(skip[:], skip_d[bl])
            return x, skip

        pre0 = load_pair(0)
        id_sb = cload("ident", id_d, [128, 128], BF16)
        wq_sb = cload("wq", wq_d, [D, HD], BF16)
        wk_sb = cload("wk", wk_d, [D, HD], BF16)
        pre1 = load_pair(1)
        wv_sb = cload("wv", wv_d, [D, HD], BF16)
        wp_sb = cload("wp", wp_d, [HD, D], FR)
        s_sb = cload("S", s_d, [128, NC, NW], BF16)
        if with_bk:
            bk_sb = cload("bk", bk_d, [HD, 1], F32)
        if with_cross:
            u_sb = cload("U", u_d, [D, H], BF16)
            g_sb = cload("gam", g_d, [H, 1], F32)
            ones_sb = const.tile([1, Q], BF16, tag="ones", name="ones")
            nc.vector.memset(ones_sb[:], 1.0)

        def phase_a(bl, pre=None):
            st_ = {}
            # ---- loads (possibly prefetched ahead of the const DMAs)
            x, skip = pre if pre is not None else load_pair(bl)

            # ---- LN stats per (token, chunk): mean/var over D.
            # bn_stats/bn_aggr reduce over the innermost axis only, so one
            # grouped op per tensor covers all 3 chunks.
            st = sb.tile([128, 3, NC, 2], F32, tag="st", name=f"st{bl}")
            bn6 = sb.tile([128, 6], F32, tag="bn6", name=f"bn6{bl}")
            for ti in range(3):
                for c in range(NC):
                    nc.vector.bn_stats(bn6[:], x[:, ti, c, :])
                    nc.vector.bn_aggr(st[:, ti, c, :], bn6[:])

            # r9 = rsqrt(var+eps): bit-trick seed on DVE, Newton on Pool
            # (gpsimd must stay SBUF-only: no PSUM access on real HW)
            v9 = sb.tile([128, 3, NC], F32, tag="v9", name=f"v9{bl}")
            nc.gpsimd.tensor_scalar_add(v9[:], st[:, :, :, 1], EPS)
            r9 = sb.tile([128, 3, NC], F32, tag="r9", name=f"r9{bl}")
            nc.vector.tensor_scalar(
                r9[:].bitcast(I32), v9[:].bitcast(I32), 1, None,
                op0=Alu.arith_shift_right,
            )
            nc.vector.tensor_scalar(
                r9[:].bitcast(I32), r9[:].bitcast(I32), -1, 0x5F3759DF,
                op0=Alu.mult, op1=Alu.add,
            )
            t9 = sb.tile([128, 3, NC], F32, tag="t9", name=f"t9{bl}")
            for _ in range(NEWTON_ITERS):
                nc.gpsimd.tensor_tensor(t9[:], r9[:], r9[:], op=Alu.mult)
                nc.gpsimd.tensor_tensor(t9[:], t9[:], v9[:], op=Alu.mult)
                nc.gpsimd.tensor_scalar(
                    t9[:], t9[:], -0.5, 1.5, op0=Alu.mult, op1=Alu.add
                )
                nc.gpsimd.tensor_tensor(r9[:], r9[:], t9[:], op=Alu.mult)

            # ---- xhat = (x - mu) * r, bf16 (SBUF-only: split DVE 4x / Pool)
            xh = {}
            nxh = 0
            for ti in range(3):
                xh[ti] = sb.tile([128, NC, D], BF16, tag=f"xh{ti}", name=f"xh{ti}_{bl}")
                for c in range(NC):
                    eng = nc.vector if nxh in (0, 1, 3, 4) else nc.gpsimd
                    eng.tensor_scalar(
                        xh[ti][:, c, :], x[:, ti, c, :],
                        st[:, ti, c, 0:1], r9[:, ti, c:c + 1],
                        op0=Alu.subtract, op1=Alu.mult,
                    )
                    nxh += 1

            # ---- q/k path first (it gates scores -> exp); v path after.
            # transposes (bf16 PSUM, bf16 identity = 1 cyc/col), DVE 2x copies
            xhT = {}
            for ti in range(2):
                tp = pa.tile([128, 1024], BF16, tag="pa", name=f"tp{ti}_{bl}")
                for c in range(NC):
                    nc.tensor.transpose(
                        tp[:, 128 * c:128 * (c + 1)], xh[ti][:, c, :], id_sb[:]
                    )
                xhT[ti] = sb.tile([D, Q], BF16, tag=f"xhT{ti}", name=f"xhT{ti}_{bl}")
                nc.vector.tensor_copy(xhT[ti][:], tp[:, 0:Q])

            qk_sb = sb.tile([HD, 2, Q], FR, tag="qk", name=f"qk{bl}")
            qpT = qk_sb[:, 0, :]
            kpT = qk_sb[:, 1, :]
            qp_ps = pa.tile([128, 512], F32, tag="pa", name=f"qp_ps{bl}")
            nc.tensor.matmul(qp_ps[:, 0:Q], wq_sb[:], xhT[0][:])
            nc.vector.tensor_copy(qpT, qp_ps[:, 0:Q])
            kp_ps = pa.tile([128, 512], F32, tag="pa", name=f"kp_ps{bl}")
            nc.tensor.matmul(kp_ps[:, 0:Q], wk_sb[:], xhT[1][:])
            if with_bk:
                nc.vector.tensor_scalar(
                    kpT, kp_ps[:, 0:Q], bk_sb[:, 0:1], None, op0=Alu.add
                )
            else:
                nc.vector.tensor_copy(kpT, kp_ps[:, 0:Q])


            if with_cross:
                # kaug[h, k] = xhat_k . U_h + gamma_h (see baseline derivation)
                ka_ps = pa.tile([128, 512], F32, tag="pa", name=f"ka_ps{bl}")
                nc.tensor.matmul(ka_ps[0:H, 0:Q], u_sb[:], xhT[1][:])
                ka = sb.tile([H, Q], BF16, tag="ka", name=f"ka{bl}")
                nc.vector.tensor_scalar(
                    ka[:], ka_ps[0:H, 0:Q], g_sb[0:H, :], None, op0=Alu.add
                )


            # ---- v path: transposes, copy, token-major projection, vp(+ones)
            tpv = pa.tile([128, 1024], BF16, tag="pa", name=f"tp2_{bl}")
            for c in range(NC):
                nc.tensor.transpose(
                    tpv[:, 128 * c:128 * (c + 1)], xh[2][:, c, :], id_sb[:]
                )
            xhT2 = sb.tile([D, Q], BF16, tag="xhT2", name=f"xhT2_{bl}")
            nc.vector.tensor_copy(xhT2[:], tpv[:, 0:Q])
            vp_ps = pa.tile([128, 512], F32, tag="pa", name=f"vp_ps{bl}")
            for c in range(NC):
                nc.tensor.matmul(
                    vp_ps[:, 128 * c:128 * (c + 1)],
                    xhT2[:, 128 * c:128 * (c + 1)], wv_sb[:],
                )
            vp = sb.tile([128, NC, H, DH + 1], BF16, tag="vp", name=f"vp{bl}", bufs=5)
            nc.gpsimd.memset(vp[:, :, :, DH:DH + 1], 1.0)
            nc.scalar.copy(
                vp[:, :, :, 0:DH],
                vp_ps[:, 0:Q].rearrange("p (c h f) -> p c h f", h=H, f=DH),
            )

            st_.update(skip=skip, qk_sb=qk_sb, vp=vp)
            if with_cross:
                st_.update(ka=ka)
            return st_

        def phase_s(bl, st_, fill=None):
            qk_sb = st_["qk_sb"]
            if with_cross:
                ka = st_["ka"]
            # ---- scores + exp per (kchunk, head-pair group)
            atts = {}
            for c in range(NC):
                if fill is not None and c > 0:
                    phase_b_av(*fill, qc=c - 1)
                for g in range(2):
                    sc_ps = scp.tile(
                        [128, 2, 512], F32, tag="sc", name=f"sc{c}{g}_{bl}"
                    )
                    for hh in range(2):
                        h = 2 * g + hh
                        nc.tensor.matmul(
                            sc_ps[:, hh, 0:Q],
                            qk_sb[32 * h:32 * (h + 1), 1, 128 * c:128 * (c + 1)],
                            qk_sb[32 * h:32 * (h + 1), 0, :],
                            tile_position=(32 * h, 0),
                            start=True, stop=not with_cross,
                        )
                        if with_cross:
                            nc.tensor.matmul(
                                sc_ps[:, hh, 0:Q],
                                ka[h:h + 1, 128 * c:128 * (c + 1)],
                                ones_sb[:],
                                start=False, stop=True,
                            )
                    att = attp.tile(
                        [128, 2, Q], BF16, tag="att", name=f"att{c}{g}_{bl}"
                    )
                    nc.scalar.activation(att[:], sc_ps[:, :, 0:Q], Exp)
                    atts[(c, g)] = att

            if fill is not None:
                phase_b_av(*fill, qc=NC - 1)
            st_.update(atts=atts)
            return st_

        def phase_b_av(bl, st_, qc):
            # ---- AV q-major with fused Z column: out[q, 33] per (qc, h).
            # Emitted as per-qc chunks interleaved between the next pair's
            # score groups so the PE never idles on the score-ring waits.
            atts, vp = st_["atts"], st_["vp"]
            if qc == 0:
                st_["av_ps"] = avp.tile(
                    [128, NC, H, DH + 1], F32, tag="av", name=f"av{bl}"
                )
            av_ps = st_["av_ps"]
            for h in range(H):
                g, hh = h // 2, h % 2
                for kc in range(NC):
                    nc.tensor.matmul(
                        av_ps[:, qc, h, :],
                        atts[(kc, g)][:, hh, 128 * qc:128 * (qc + 1)],
                        vp[:, kc, h, :],
                        start=(kc == 0), stop=(kc == 2),
                    )

        def phase_b(bl, st_):
            skip = st_["skip"]
            av_ps = st_["av_ps"]
            # ---- normalize by 1/Z (per-partition in q-major layout): one
            # broadcast multiply straight out of PSUM on DVE
            zi = sb.tile([128, NC, H], F32, tag="zi", name=f"zi{bl}")
            nc.vector.reciprocal(zi[:], av_ps[:, :, :, DH])
            avn = sb.tile([128, NC, H, DH], BF16, tag="avn", name=f"avn{bl}")
            for qc in range(NC):
                zi_bc = zi[:, qc, :].unsqueeze(-1).broadcast_to([128, H, DH])
                nc.vector.tensor_tensor(
                    avn[:, qc, :, :], av_ps[:, qc, :, 0:DH], zi_bc, op=Alu.mult
                )
            # ---- view-mean via S-matmul: avm[hd, w] += avn[:,qc,:,:]^T @ S_qc
            avm_ps = pa.tile([128, 512], F32, tag="pa", name=f"avm_ps{bl}")
            for qc in range(NC):
                nc.tensor.matmul(
                    avm_ps[:, 0:NW], avn[:, qc, :, :], s_sb[:, qc, :],
                    start=(qc == 0), stop=(qc == 2),
                )

            avm = sb.tile([HD, NW], FR, tag="avm", name=f"avm{bl}")
            nc.vector.tensor_copy(avm[:], avm_ps[:, 0:NW])
            # ---- output projection, skip add, store
            z_ps = pa.tile([128, 512], F32, tag="pa", name=f"z_ps{bl}")
            nc.tensor.matmul(z_ps[:, 0:NW], wp_sb[:], avm[:])
            zo = sb.tile([D, NW], F32, tag="zo", name=f"zo{bl}")
            nc.vector.tensor_tensor(zo[:], z_ps[:, 0:NW], skip[:], op=Alu.add)
            # issue the store from the DVE queue: the zo-ready wait is already
            # satisfied there (in-order after the add), keeping SP free for loads
            nc.sync.dma_start(out_d[bl], zo[:])

        # 2-stage software pipeline: [A](k), then B(k-2) fills the PE gaps
        # left by score-group ring waits, then S(k) (scores+exp).
        stA = {}
        pres = {0: pre0, 1: pre1}
        for bl in range(PER_CORE):
            stA[bl] = phase_a(bl, pres.pop(bl, None))
            fill = (bl - 2, stA[bl - 2]) if bl - 2 in stA else None
            phase_s(bl, stA[bl], fill=fill)
            if bl - 2 in stA:
                phase_b(bl - 2, stA.pop(bl - 2))
        for bl in (PER_CORE - 2, PER_CORE - 1):
            for qc in range(NC):
                phase_b_av(bl, stA[bl], qc=qc)
            phase_b(bl, stA.pop(bl))


def _build(with_bk, with_cross):
    key = (bool(with_bk), bool(with_cross))
    if key in _COMPILED:
        return _COMPILED[key]
    nc = bacc.Bacc("TRN2", target_bir_lowering=False, debug=False)
    _emit(nc, *key)
    nc.compile()
    _COMPILED[key] = nc
    return nc


def _prep_host(inputs):
    q, k, v, skip = inputs["q"], inputs["k"], inputs["v"], inputs["skip"]
    scale = np.float32(DH ** -0.5)
    fold = lambda t: np.ascontiguousarray(
        t.transpose(0, 2, 3, 1, 4, 5, 6).reshape(BL, Q, D)
    )
    X = np.stack([fold(q), fold(k), fold(v)], axis=1).astype(ml_dtypes.bfloat16)
    wq = (inputs["lnq_g"][:, None] * inputs["wq"] * scale).astype(ml_dtypes.bfloat16)
    wk = (inputs["lnk_g"][:, None] * inputs["wk"]).astype(np.float32)  # bf16 below; f32 for U
    wv = (inputs["lnv_g"][:, None] * inputs["wv"]).astype(ml_dtypes.bfloat16)
    wp = (inputs["wp"] / 6.0).astype(np.float32)
    bkp = (inputs["lnk_b"] @ inputs["wk"] + inputs["bk"]).astype(np.float32)
    bqp = ((inputs["lnq_b"] @ inputs["wq"] + inputs["bq"]) * scale).astype(np.float32)
    bvp = (inputs["lnv_b"] @ inputs["wv"] + inputs["bv"]).astype(np.float32)
    skipT = np.ascontiguousarray(
        (skip.reshape(BL, NW, D) + inputs["bp"] + bvp @ inputs["wp"])
        .transpose(0, 2, 1)
    ).astype(np.float32)
    # view-mean S matrix: token t = 3p + qc -> window position w = t % 64
    p_idx = np.arange(128)
    S = np.zeros((128, NC, NW), np.float32)
    for qc in range(NC):
        S[p_idx, qc, (3 * p_idx + qc) % NW] = 1.0
    S = S.astype(ml_dtypes.bfloat16)
    ident = np.eye(128, dtype=np.float32).astype(ml_dtypes.bfloat16)
    # q-side bias: softmax-invariant part drops; k-dependent cross term uses
    # U[:, h] = wk'_hblock @ bqp_hblock and gamma_h = bk'_h . bqp_h
    U = np.zeros((D, H), np.float32)
    gam = np.zeros((H, 1), np.float32)
    for h in range(H):
        s = slice(h * DH, (h + 1) * DH)
        U[:, h] = wk[:, s] @ bqp[s]
        gam[h, 0] = bkp[s] @ bqp[s]
    with_bk = bool(np.abs(bkp).max() > 0)
    with_cross = bool(np.abs(bqp).max() > 0)
    consts = dict(
        wq=wq, wk=wk.astype(ml_dtypes.bfloat16), wv=wv, wp=wp, S=S, ident=ident,
        bk=bkp.reshape(HD, 1), U=U.astype(ml_dtypes.bfloat16), gam=gam,
    )
    in_maps = []
    for c in range(NCORES):
        s = slice(c * PER_CORE, (c + 1) * PER_CORE)
        m = dict(
            X=np.ascontiguousarray(X[s]),
            skipT=np.ascontiguousarray(skipT[s]),
        )
        m.update({k_: v_.copy() for k_, v_ in consts.items()})
        in_maps.append(m)
    return in_maps, with_bk, with_cross


def kernel(**inputs):
    inputs = {k: np.asarray(v, dtype=np.float32) for k, v in inputs.items()}
    in_maps, with_bk, with_cross = _prep_host(inputs)
    nc = _build(with_bk, with_cross)
    res = run_bass_kernel_spmd(nc, in_maps, list(range(NCORES)))
    zT = np.concatenate([r["out"] for r in res.results], axis=0)  # [BL, D, 64]
    z = zT.transpose(0, 2, 1).reshape(B, X, Y, W1, W2, D)
    return np.ascontiguousarray(z)
